# revision 2
# baseline (speedup 1.0000x reference)
"""Trainium2 Bass kernel for nn_CustomDense (bit-serial quantized dense layer).

Math: for bits=8 the reference's per-element bit-serial loop computes exactly
    f(x, w) = trunc(x * w / 256)        (x in [0,15], w in [-128,127])
so  out = relu(sum_d f(x_bd, w_du) + bias_u).

Key identity: trunc(1*w/256) = 0 for all w in [-128,127], so the v=1 term
vanishes and
    sum_d f = sum_{v=2..15} H_v^T @ T_v
with H_v[d,b] = 1[x[b,d]==v] and T_v[d,u] = trunc(v*w[d,u]/256) in [-8,7].
No sign/divisibility corrections are needed in the trunc basis.

Both operand families are small integers, exactly representable in fp8
(e4m3): masks are 0/1, tables are in [-8,7]. The host precomputes all 28
operand tensors per core; the device does only fp8 matmuls (fp32 PSUM,
integer-exact), a PSUM->SBUF cast to fp16 (partials are integers |.|<=1024,
fp16-exact), and a DMA out. Host sums the 8 partials, adds bias, relu --
bit-identical to the reference.

Sharding: D (contraction, 1024) split across 8 cores, 128 rows each.
"""

import numpy as np

B, D, U, BITS = 64, 1024, 1024, 8
NCORES = 8
DSH = D // NCORES  # 128 contraction rows per core
NV = 14            # groups: v = 2..15
TRACE = False

_NC_CACHE = {}


def _build_nc():
    import concourse.bacc as bacc
    import concourse.mybir as mybir
    import concourse.tile as tile

    f8 = mybir.dt.float8e4
    f16 = mybir.dt.float16

    nc = bacc.Bacc("TRN2", target_bir_lowering=False, debug=False)
    tt_d = nc.dram_tensor("tt", [DSH, NV, U], f8, kind="ExternalInput")
    hm_d = nc.dram_tensor("hm", [DSH, NV, B], f8, kind="ExternalInput")
    out_d = nc.dram_tensor("out", [2, B, 512], f16, kind="ExternalOutput")

    with tile.TileContext(nc) as tc:
        with (
            tc.tile_pool(name="io", bufs=1) as io,
            tc.tile_pool(name="ps", bufs=1, space="PSUM") as ps,
        ):
            hm_sb = io.tile([DSH, NV, B], f8)
            tt0_sb = io.tile([DSH, NV // 2, U], f8, tag="tt0")
            tt1_sb = io.tile([DSH, NV - NV // 2, U], f8, tag="tt1")
            nc.scalar.dma_start(hm_sb[:], hm_d[:])
            nc.sync.dma_start(tt0_sb[:], tt_d[:, 0:NV // 2])
            nc.sync.dma_start(tt1_sb[:], tt_d[:, NV // 2:NV])

            acc0 = ps.tile([B, 512], mybir.dt.float32, tag="acc0")
            acc1 = ps.tile([B, 512], mybir.dt.float32, tag="acc1")

            def rhs(j, half):
                t = tt0_sb if j < NV // 2 else tt1_sb
                jj = j if j < NV // 2 else j - NV // 2
                return t[:, jj, half * 512:(half + 1) * 512]

            # all h0 groups first (drain acc0 early), then h1
            for j in range(NV):
                nc.tensor.matmul(
                    acc0[:], hm_sb[:, j, :], rhs(j, 0),
                    start=(j == 0), stop=(j == NV - 1),
                )
            o_sb = io.tile([B, 1024], f16, tag="osb")
            nc.vector.tensor_copy(o_sb[:, 0:512], acc0[:])
            nc.sync.dma_start(out_d[0], o_sb[:, 0:512])
            for j in range(NV):
                nc.tensor.matmul(
                    acc1[:], hm_sb[:, j, :], rhs(j, 1),
                    start=(j == 0), stop=(j == NV - 1),
                )
            nc.vector.tensor_copy(o_sb[:, 512:1024], acc1[:])
            nc.scalar.dma_start(out_d[1], o_sb[:, 512:1024])

    nc.compile()
    return nc


def _get_nc():
    if "nc" not in _NC_CACHE:
        _NC_CACHE["nc"] = _build_nc()
    return _NC_CACHE["nc"]


_LAST_RESULTS = {}


def _kernel_numpy(inputs, bits, kernel, bias):
    # generic (non-8-bit) fallback; mirrors the reference exactly
    x = np.asarray(inputs, np.float64)
    w = np.asarray(kernel, np.float64)
    b = int(bits)
    out = np.zeros((x.shape[0], w.shape[1]), np.float64)
    scale = float(2 ** b)
    for d0 in range(0, w.shape[0], 128):
        d1 = min(d0 + 128, w.shape[0])
        wm = np.sign(w[None, d0:d1, :]) * (
            np.abs(w[None, d0:d1, :]) % scale if b < 31 else np.abs(w[None, d0:d1, :])
        )
        out += np.trunc(x[:, d0:d1, None] * wm / scale).sum(1)
    return np.maximum(out + np.asarray(bias, np.float64)[None, :], 0.0).astype(
        np.float32
    )


def kernel(inputs, bits, kernel, bias):
    if int(bits) != BITS:
        return _kernel_numpy(inputs, bits, kernel, bias)

    import ml_dtypes
    from concourse.bass_utils import run_bass_kernel_spmd

    x = np.asarray(inputs)
    w = np.asarray(kernel)
    b = np.asarray(bias, dtype=np.float32)
    assert x.shape == (B, D) and w.shape == (D, U)

    f8 = ml_dtypes.float8_e4m3
    xt = x.T.astype(np.int32)                    # [D, B]
    wi = w.astype(np.int32)                      # [D, U]

    # tables T_v = trunc(v*w/256) in [-8,7]; masks H_v = (x==v), v=2..15
    tt = np.empty((D, NV, U), dtype=f8)
    hm = np.empty((D, NV, B), dtype=f8)
    for j in range(NV):
        v = j + 2
        tt[:, j, :] = ((v * wi) // 256 + ((wi < 0) & ((v * wi) % 256 != 0))).astype(f8)
        hm[:, j, :] = (xt == v).astype(f8)

    in_maps = [
        {
            "tt": np.ascontiguousarray(tt[c * DSH:(c + 1) * DSH]),
            "hm": np.ascontiguousarray(hm[c * DSH:(c + 1) * DSH]),
        }
        for c in range(NCORES)
    ]

    nc = _get_nc()
    res = run_bass_kernel_spmd(
        nc, in_maps, core_ids=list(range(NCORES)), trace=TRACE
    )
    _LAST_RESULTS["res"] = res

    total = np.zeros((B, U), dtype=np.float32)
    for r in res.results:
        o = r["out"].astype(np.float32)
        total[:, 0:512] += o[0]
        total[:, 512:1024] += o[1]
    return np.maximum(total + b[None, :], 0.0).astype(np.float32)
